# revision 46
# baseline (speedup 1.0000x reference)
"""Sparse 3D conv (gather -> per-offset matmul -> accumulate) on 8 TRN2 NeuronCores.

Strategy (data-parallel over output voxels, per the sharding hint):
  - Shard the N=200000 output voxels across 8 cores (25000 each, padded to
    25088 = 49 compute tiles x 512 voxels, streamed as 25 DMA blocks of
    1024 voxels).
  - Host-side sharding prep builds each core's operand stream in the layout
    the PE consumes directly: per 1024-voxel block a [128, 7, 1024] fp8-E3M4
    array whose partition rows are (k%4)*32+ch for kernel-offset group
    g=k//4 (27 offsets padded to 28 = 7 groups of 4) and whose columns are
    the voxels, with masked/padded entries zeroed. fp8 E3M4 keeps int8-level
    accuracy (4 mantissa bits, rel err 1.26e-2 vs int8's 1.12e-2) while
    letting the PE read the stream directly against bf16 weights — no cast
    DMA, so the stream moves at the HBM line rate (~420 GB/s burst).
  - Replicate the small [128, 7*64] grouped bf16 weight stack to every core.
  - Device per block: one plain 917KB HWDGE DMA on the SP ring (the input
    stream owns that ring from the first instruction — weights ride the ACT
    ring; the last block is packed compactly and split at a group boundary
    for a short final drain chain). Per 512-voxel tile, 7 pairs
    of col-tiled PE matmuls: voxels 0:256 run in PE column-half 0 (PSUM
    partitions 0:64) while voxels 256:512 run concurrently in column-half 1
    (PSUM partitions 64:128), accumulating over the 7 groups (pair slot
    ~109ns -> 765ns/tile, PE is never the pacer). Copy [128,256] f32 -> fp16
    into a 4-tile buffer; one [128, 2KB] DMA per 4 tiles to a per-core
    [128, 49*256] fp16 buffer (host de-interleaves).
    Memory-bound: ~22.5MB/core HBM read + 3.2MB write at line rate.
  - Host transposes/crops/concats per-core outputs to the full [200000, 64].
"""
import numpy as np
import ml_dtypes
from contextlib import ExitStack

import concourse.bass as bass
import concourse.bacc as bacc
import concourse.mybir as mybir
import concourse.tile as tile
from concourse.bass_utils import run_bass_kernel_spmd

N = 200000
K = 27
CIN = 32
COUT = 64
NCORES = 8
PERCORE = N // NCORES          # 25000
VTILE = 512
HALF = VTILE // 2              # 256
NTILES = (PERCORE + VTILE - 1) // VTILE   # 49 (512-voxel compute tiles)
NPAD = NTILES * VTILE          # 25088
NGRP = 7                       # 28 k-slots (27 real + 1 zero) in groups of 4
OBATCH = 4                     # tiles per output DMA
DSUB = 2                       # compute tiles per DMA block
DTILE = DSUB * VTILE           # 1024-voxel DMA blocks (917KB per transfer)
NDTILES = (NTILES + DSUB - 1) // DSUB   # 25; block 24 only has tile 48
DPAD = NDTILES * DTILE         # 25600 (padding cols are never transferred)

_NC_CACHE = None


def _build_kernel():
    nc = bacc.Bacc("TRN2", target_bir_lowering=False)
    gts = nc.dram_tensor("gts", [NDTILES, 128, NGRP * DTILE], mybir.dt.float8e3,
                         kind="ExternalInput")
    wst = nc.dram_tensor("wst", [128, NGRP * COUT], mybir.dt.bfloat16,
                         kind="ExternalInput")
    outT = nc.dram_tensor("outT", [128, NTILES * HALF], mybir.dt.float16,
                          kind="ExternalOutput")

    with tile.TileContext(nc) as tc, ExitStack() as ctx:
        const = ctx.enter_context(tc.tile_pool(name="const", bufs=1))
        sbg = ctx.enter_context(tc.tile_pool(name="sbg", bufs=8))
        sbo = ctx.enter_context(tc.tile_pool(name="sbo", bufs=3))
        opsum = ctx.enter_context(tc.tile_pool(name="opsum", bufs=4, space="PSUM"))

        # weights ride the ACT ring so the input stream owns the SP ring
        # from the first instruction
        w_sb = const.tile([128, NGRP * COUT], mybir.dt.bfloat16, name="w_sb")
        nc.scalar.dma_start(w_sb[:], wst[:])

        out_sb = None
        gt = None
        for t in range(NTILES):
            p, sub = divmod(t, DSUB)
            last_blk = p == NDTILES - 1
            if sub == 0:
                # one 917KB HBM->SBUF transfer covers two 512-voxel compute
                # tiles; the layout inside is [(j,ch), g, col(1024)]. The
                # final block holds only tile 48, packed compactly by the
                # host as [(j,ch), g, col(512)]. First/last blocks are split
                # at a group boundary (contiguous descriptors) so compute
                # starts earlier / the final drain chain is short.
                w = VTILE if last_blk else DTILE
                gt = sbg.tile([128, NGRP, w], mybir.dt.float8e3,
                              name="gt", tag="gt")
                gsrc = gts[p][:, :NGRP * w].rearrange("q (g v) -> q g v",
                                                      g=NGRP)
                if last_blk:
                    nc.sync.dma_start(out=gt[:, 0:4, :], in_=gsrc[:, 0:4, :])
                    nc.sync.dma_start(out=gt[:, 4:NGRP, :],
                                      in_=gsrc[:, 4:NGRP, :])
                else:
                    nc.sync.dma_start(out=gt[:], in_=gsrc)

            # col-tiled pair: voxels [0:256) -> PE col-half 0 / PSUM 0:64,
            # voxels [256:512) -> PE col-half 1 / PSUM 64:128; both streams
            # run concurrently in the two column halves of the PE array.
            out_p = opsum.tile([128, HALF], mybir.dt.float32,
                               name="out_p", tag="op")
            for g in range(NGRP):
                w_g = w_sb[:, g * COUT:(g + 1) * COUT]
                base = 0 if last_blk else sub * VTILE
                rhs_a = gt[:, g, base:base + HALF]
                rhs_b = gt[:, g, base + HALF:base + VTILE]
                nc.tensor.matmul(
                    out_p[0:COUT, :], lhsT=w_g, rhs=rhs_a,
                    start=(g == 0), stop=(g == NGRP - 1),
                )
                nc.tensor.matmul(
                    out_p[COUT:128, :], lhsT=w_g, rhs=rhs_b,
                    start=(g == 0), stop=(g == NGRP - 1),
                )

            bslot = t % OBATCH
            if bslot == 0:
                ob_lo = t                      # first tile in this batch
                nb = min(OBATCH, NTILES - t)   # tiles in this batch
                out_sb = sbo.tile([128, OBATCH * HALF], mybir.dt.float16,
                                  name="out_sb", tag="ob")
            dst = out_sb[:, bslot * HALF:(bslot + 1) * HALF]
            if t % 2 == 0:
                nc.vector.tensor_copy(dst, out_p[:])
            else:
                nc.scalar.copy(dst, out_p[:])
            if bslot == nb - 1:
                nc.scalar.dma_start(
                    outT[:, ob_lo * HALF:(ob_lo + nb) * HALF],
                    out_sb[:, :nb * HALF])

    nc.compile()
    return nc


def _get_nc():
    global _NC_CACHE
    if _NC_CACHE is None:
        _NC_CACHE = _build_kernel()
    return _NC_CACHE


def _prep_host(features, neighbor_map, neighbor_mask, kernel):
    """Build per-core device inputs.

    Returns (gts_all, gts6_all, wst): gts_all[c] is [NTILES, 128, 6*VTILE]
    fp8e3 with partition row (k%4)*32+ch of group k//4 holding channel ch of
    the voxel's k-th gathered neighbor (features scaled by 15.5/max|f|);
    gts6_all[c] is the [NTILES, 96, VTILE] block for k=24..26; wst is the
    matching [128, NGRP*COUT] bf16 weight stack with the scale divided out.
    """
    feat = np.asarray(features, dtype=np.float32)
    scale = 15.5 / max(np.abs(feat).max(), 1e-30)
    featq = (feat * scale).astype(ml_dtypes.float8_e3m4).view(np.uint8)
    feat_ext = np.vstack([featq, np.zeros((1, CIN), dtype=np.uint8)])
    nm = np.asarray(neighbor_map, dtype=np.int64)      # [27, N]
    mk = np.asarray(neighbor_mask, dtype=bool)          # [27, N]

    # weight stack: group g partition rows 32j..32j+31 = kernel[4g+j]
    w = np.asarray(kernel, dtype=np.float32)            # [27, 32, 64]
    wstk = np.zeros((NGRP, 4, CIN, COUT), dtype=np.float32)
    for g in range(NGRP):
        for j in range(4):
            kk = 4 * g + j
            if kk < K:
                wstk[g, j] = w[kk]
    wst = np.ascontiguousarray(
        wstk.transpose(1, 2, 0, 3).reshape(128, NGRP * COUT) / scale
    ).astype(ml_dtypes.bfloat16)

    gts_all = []
    for c in range(NCORES):
        vloc = np.arange(DPAD)
        vglob = np.minimum(c * PERCORE + vloc, N - 1)
        valid_v = vloc < PERCORE                        # [DPAD]
        nmv = nm[:, vglob]                              # [27, DPAD]
        mskv = mk[:, vglob] & valid_v[None, :]
        src = np.where(mskv, nmv, N)                    # masked -> zero row
        g27 = feat_ext[src]                             # [27, DPAD, 32] u8
        g28 = np.zeros((NGRP * 4, DPAD, CIN), dtype=np.uint8)
        g28[:K] = g27
        # [28=(g,j), DPAD=(p,col), ch] -> [p, (j, ch), g, col]
        g28 = g28.reshape(NGRP, 4, NDTILES, DTILE, CIN)
        gt = g28.transpose(2, 1, 4, 0, 3).reshape(NDTILES, 128, NGRP * DTILE)
        gt = np.ascontiguousarray(gt)
        # repack the final block (tile 48 only) compactly: [(j,ch), g, 512]
        lastc = gt[NDTILES - 1].reshape(128, NGRP, DTILE)[:, :, :VTILE]
        gt[NDTILES - 1, :, :NGRP * VTILE] = lastc.reshape(128, NGRP * VTILE)
        gts_all.append(gt.view(ml_dtypes.float8_e3m4))
    return gts_all, wst



def _postprocess(res):
    outs = []
    for c in range(NCORES):
        oT = np.asarray(res.results[c]["outT"], dtype=np.float32)
        # [128, NTILES*HALF]: row h*64+c, col t*HALF+j  ->  voxel
        # t*VTILE + h*HALF + j, channel c
        o = oT.reshape(2, COUT, NTILES, HALF).transpose(2, 0, 3, 1)
        outs.append(o.reshape(NPAD, COUT)[:PERCORE])    # [25000, 64]
    return np.concatenate(outs, axis=0).astype(np.float32)


def kernel(features, neighbor_map, neighbor_mask, kernel):
    gts_all, wst = _prep_host(features, neighbor_map, neighbor_mask, kernel)
    nc = _get_nc()
    in_maps = [{"gts": gts_all[c], "wst": wst} for c in range(NCORES)]
    res = run_bass_kernel_spmd(nc, in_maps, core_ids=list(range(NCORES)))
    return _postprocess(res)


# revision 47
# speedup vs baseline: 1.0218x; 1.0218x over previous
"""Sparse 3D conv (gather -> per-offset matmul -> accumulate) on 8 TRN2 NeuronCores.

Strategy (data-parallel over output voxels, per the sharding hint):
  - Shard the N=200000 output voxels across 8 cores (25000 each, padded to
    25088 = 49 compute tiles x 512 voxels, streamed as 25 DMA blocks of
    1024 voxels).
  - Host-side sharding prep builds each core's operand stream in the layout
    the PE consumes directly: per 1024-voxel block a [128, 7, 1024] fp8-E3M4
    array whose partition rows are (k%4)*32+ch for kernel-offset group
    g=k//4 (27 offsets padded to 28 = 7 groups of 4) and whose columns are
    the voxels, with masked/padded entries zeroed. fp8 E3M4 keeps int8-level
    accuracy (4 mantissa bits, rel err 1.26e-2 vs int8's 1.12e-2) while
    letting the PE read the stream directly against bf16 weights — no cast
    DMA, so the stream moves at the HBM line rate (~420 GB/s burst).
  - Replicate the small [128, 7*64] grouped bf16 weight stack to every core.
  - Device per block: one plain 917KB HWDGE DMA on the SP ring (the input
    stream owns that ring from the first instruction — weights ride the ACT
    ring; the last block is packed compactly and split at a group boundary
    for a short final drain chain). Per 512-voxel tile, 7 pairs
    of col-tiled PE matmuls: voxels 0:256 run in PE column-half 0 (PSUM
    partitions 0:64) while voxels 256:512 run concurrently in column-half 1
    (PSUM partitions 64:128), accumulating over the 7 groups (pair slot
    ~109ns -> 765ns/tile, PE is never the pacer). Copy [128,256] f32 -> fp16
    into a 4-tile buffer; one [128, 2KB] DMA per 4 tiles to a per-core
    [128, 49*256] fp16 buffer (host de-interleaves).
    Memory-bound: ~22.5MB/core HBM read + 3.2MB write at line rate.
  - Host transposes/crops/concats per-core outputs to the full [200000, 64].
"""
import numpy as np
import ml_dtypes
from contextlib import ExitStack

import concourse.bass as bass
import concourse.bacc as bacc
import concourse.mybir as mybir
import concourse.tile as tile
from concourse.bass_utils import run_bass_kernel_spmd

N = 200000
K = 27
CIN = 32
COUT = 64
NCORES = 8
PERCORE = N // NCORES          # 25000
VTILE = 512
HALF = VTILE // 2              # 256
NTILES = (PERCORE + VTILE - 1) // VTILE   # 49 (512-voxel compute tiles)
NPAD = NTILES * VTILE          # 25088
NGRP = 7                       # 28 k-slots (27 real + 1 zero) in groups of 4
OBATCH = 4                     # tiles per output DMA
DSUB = 2                       # compute tiles per DMA block
DTILE = DSUB * VTILE           # 1024-voxel DMA blocks (917KB per transfer)
NDTILES = (NTILES + DSUB - 1) // DSUB   # 25; block 24 only has tile 48
DPAD = NDTILES * DTILE         # 25600 (padding cols are never transferred)

_NC_CACHE = None


def _build_kernel():
    nc = bacc.Bacc("TRN2", target_bir_lowering=False)
    gts = nc.dram_tensor("gts", [NDTILES, 128, NGRP * DTILE], mybir.dt.float8e3,
                         kind="ExternalInput")
    wst = nc.dram_tensor("wst", [128, NGRP * COUT], mybir.dt.bfloat16,
                         kind="ExternalInput")
    outT = nc.dram_tensor("outT", [128, NTILES * HALF], mybir.dt.float16,
                          kind="ExternalOutput")

    with tile.TileContext(nc) as tc, ExitStack() as ctx:
        const = ctx.enter_context(tc.tile_pool(name="const", bufs=1))
        sbg = ctx.enter_context(tc.tile_pool(name="sbg", bufs=6))
        sbo = ctx.enter_context(tc.tile_pool(name="sbo", bufs=3))
        opsum = ctx.enter_context(tc.tile_pool(name="opsum", bufs=4, space="PSUM"))

        # weights ride the ACT ring so the input stream owns the SP ring
        # from the first instruction
        w_sb = const.tile([128, NGRP * COUT], mybir.dt.bfloat16, name="w_sb")
        nc.scalar.dma_start(w_sb[:], wst[:])

        out_sb = None
        gt = None
        for t in range(NTILES):
            p, sub = divmod(t, DSUB)
            last_blk = p == NDTILES - 1
            if sub == 0:
                # one 917KB HBM->SBUF transfer covers two 512-voxel compute
                # tiles; the layout inside is [(j,ch), g, col(1024)]. The
                # final block holds only tile 48, packed compactly by the
                # host as [(j,ch), g, col(512)]. First/last blocks are split
                # at a group boundary (contiguous descriptors) so compute
                # starts earlier / the final drain chain is short.
                w = VTILE if last_blk else DTILE
                gt = sbg.tile([128, NGRP, w], mybir.dt.float8e3,
                              name="gt", tag="gt")
                gsrc = gts[p][:, :NGRP * w].rearrange("q (g v) -> q g v",
                                                      g=NGRP)
                if last_blk:
                    nc.sync.dma_start(out=gt[:, 0:4, :], in_=gsrc[:, 0:4, :])
                    nc.sync.dma_start(out=gt[:, 4:NGRP, :],
                                      in_=gsrc[:, 4:NGRP, :])
                else:
                    nc.sync.dma_start(out=gt[:], in_=gsrc)

            # col-tiled pair: voxels [0:256) -> PE col-half 0 / PSUM 0:64,
            # voxels [256:512) -> PE col-half 1 / PSUM 64:128; both streams
            # run concurrently in the two column halves of the PE array.
            out_p = opsum.tile([128, HALF], mybir.dt.float32,
                               name="out_p", tag="op")
            for g in range(NGRP):
                w_g = w_sb[:, g * COUT:(g + 1) * COUT]
                base = 0 if last_blk else sub * VTILE
                rhs_a = gt[:, g, base:base + HALF]
                rhs_b = gt[:, g, base + HALF:base + VTILE]
                nc.tensor.matmul(
                    out_p[0:COUT, :], lhsT=w_g, rhs=rhs_a,
                    start=(g == 0), stop=(g == NGRP - 1),
                )
                nc.tensor.matmul(
                    out_p[COUT:128, :], lhsT=w_g, rhs=rhs_b,
                    start=(g == 0), stop=(g == NGRP - 1),
                )

            bslot = t % OBATCH
            if bslot == 0:
                ob_lo = t                      # first tile in this batch
                nb = min(OBATCH, NTILES - t)   # tiles in this batch
                out_sb = sbo.tile([128, OBATCH * HALF], mybir.dt.float16,
                                  name="out_sb", tag="ob")
            dst = out_sb[:, bslot * HALF:(bslot + 1) * HALF]
            if t % 2 == 0:
                nc.vector.tensor_copy(dst, out_p[:])
            else:
                nc.scalar.copy(dst, out_p[:])
            if bslot == nb - 1:
                nc.scalar.dma_start(
                    outT[:, ob_lo * HALF:(ob_lo + nb) * HALF],
                    out_sb[:, :nb * HALF])

    nc.compile()
    return nc


def _get_nc():
    global _NC_CACHE
    if _NC_CACHE is None:
        _NC_CACHE = _build_kernel()
    return _NC_CACHE


def _prep_host(features, neighbor_map, neighbor_mask, kernel):
    """Build per-core device inputs.

    Returns (gts_all, gts6_all, wst): gts_all[c] is [NTILES, 128, 6*VTILE]
    fp8e3 with partition row (k%4)*32+ch of group k//4 holding channel ch of
    the voxel's k-th gathered neighbor (features scaled by 15.5/max|f|);
    gts6_all[c] is the [NTILES, 96, VTILE] block for k=24..26; wst is the
    matching [128, NGRP*COUT] bf16 weight stack with the scale divided out.
    """
    feat = np.asarray(features, dtype=np.float32)
    scale = 15.5 / max(np.abs(feat).max(), 1e-30)
    featq = (feat * scale).astype(ml_dtypes.float8_e3m4).view(np.uint8)
    feat_ext = np.vstack([featq, np.zeros((1, CIN), dtype=np.uint8)])
    nm = np.asarray(neighbor_map, dtype=np.int64)      # [27, N]
    mk = np.asarray(neighbor_mask, dtype=bool)          # [27, N]

    # weight stack: group g partition rows 32j..32j+31 = kernel[4g+j]
    w = np.asarray(kernel, dtype=np.float32)            # [27, 32, 64]
    wstk = np.zeros((NGRP, 4, CIN, COUT), dtype=np.float32)
    for g in range(NGRP):
        for j in range(4):
            kk = 4 * g + j
            if kk < K:
                wstk[g, j] = w[kk]
    wst = np.ascontiguousarray(
        wstk.transpose(1, 2, 0, 3).reshape(128, NGRP * COUT) / scale
    ).astype(ml_dtypes.bfloat16)

    gts_all = []
    for c in range(NCORES):
        vloc = np.arange(DPAD)
        vglob = np.minimum(c * PERCORE + vloc, N - 1)
        valid_v = vloc < PERCORE                        # [DPAD]
        nmv = nm[:, vglob]                              # [27, DPAD]
        mskv = mk[:, vglob] & valid_v[None, :]
        src = np.where(mskv, nmv, N)                    # masked -> zero row
        g27 = feat_ext[src]                             # [27, DPAD, 32] u8
        g28 = np.zeros((NGRP * 4, DPAD, CIN), dtype=np.uint8)
        g28[:K] = g27
        # [28=(g,j), DPAD=(p,col), ch] -> [p, (j, ch), g, col]
        g28 = g28.reshape(NGRP, 4, NDTILES, DTILE, CIN)
        gt = g28.transpose(2, 1, 4, 0, 3).reshape(NDTILES, 128, NGRP * DTILE)
        gt = np.ascontiguousarray(gt)
        # repack the final block (tile 48 only) compactly: [(j,ch), g, 512]
        lastc = gt[NDTILES - 1].reshape(128, NGRP, DTILE)[:, :, :VTILE]
        gt[NDTILES - 1, :, :NGRP * VTILE] = lastc.reshape(128, NGRP * VTILE)
        gts_all.append(gt.view(ml_dtypes.float8_e3m4))
    return gts_all, wst



def _postprocess(res):
    outs = []
    for c in range(NCORES):
        oT = np.asarray(res.results[c]["outT"], dtype=np.float32)
        # [128, NTILES*HALF]: row h*64+c, col t*HALF+j  ->  voxel
        # t*VTILE + h*HALF + j, channel c
        o = oT.reshape(2, COUT, NTILES, HALF).transpose(2, 0, 3, 1)
        outs.append(o.reshape(NPAD, COUT)[:PERCORE])    # [25000, 64]
    return np.concatenate(outs, axis=0).astype(np.float32)


def kernel(features, neighbor_map, neighbor_mask, kernel):
    gts_all, wst = _prep_host(features, neighbor_map, neighbor_mask, kernel)
    nc = _get_nc()
    in_maps = [{"gts": gts_all[c], "wst": wst} for c in range(NCORES)]
    res = run_bass_kernel_spmd(nc, in_maps, core_ids=list(range(NCORES)))
    return _postprocess(res)
